# revision 25
# baseline (speedup 1.0000x reference)
"""ExtractSearchWindows Trainium2 kernel.

Math (search_range=3, template=7):
  out[b,i,j,dy*7+dx,ty*7+tx] = u8(floor(Qpad[b, i+dy+ty, j+dx+tx]))
with Qpad = zero-pad(x[:,0], 6) of shape (2, 204, 204), out (2,192,192,49,49) u8.

Strategy (8 cores, data-parallel over the 384 (b,i) output rows, 48 rows/core):
  Host: floor+cast+pad the tiny input; per core build QEL[54, 8*204] u8 where
        QEL[p, r*204+col] = Qpad[b, i0+p+r, col]  (p = rbase, r = u+ty)
  Device per core, split into 3 j-chunks (one owner engine per chunk so every
  out-DMA needs at most one cross-engine wait — HW allows 1 wait/instruction):
    stage 1: 98 engine copies per chunk (u,ty,dx) expand QEL into the paired
             slab L2[54, jn*686]:
               L2[p][j, u*343 + dx*49 + ty*7 + tx] = Qpad[p+u+ty, j+dx+tx]
             i.e. L2[p][j] = [block(rbase=p) | block(rbase=p+1)]  (~7 MB)
    stage 2: per output row i, overlapping-read DMAs with >=512B descriptors:
               A: dy in {0..5}: src L2[i],L2[i+2],L2[i+4] (partition step 2),
                  686B contiguous runs -> out[i][j, dy*343...]
               B: dy=6: src L2[i+6][j, 0:343] -> out[i][j, 6*343...]
  Host: stitch the 8 per-core (48,192,49,49) chunks into the full output.
"""
import sys

sys.path.insert(0, "/opt/trn_rl_repo")

import numpy as np

TEMPLATE = 7
MAX_SR = 3
H = W = 192
PAD = MAX_SR + TEMPLATE // 2          # 6
PADW = W + 2 * PAD                    # 204
CV = 7                                # 2*search_range+1
BLK = CV * TEMPLATE * TEMPLATE        # 343 bytes: (dx,ty,tx) block
PBLK = 2 * BLK                        # 686: paired (u, dx,ty,tx) block
ROWBLK = CV * BLK                     # 2401 bytes: (dy,dx,ty,tx) block
NI = 48                               # output rows per core
NR = NI + CV - 1                      # 54 rbase partitions
NWIN = TEMPLATE + 1                   # 8 rows per QEL partition (r = u+ty)
QELFREE = NWIN * PADW                 # 1632
N_CORES = 8

# Slice schedule: (name, engine, issuer_ring, jn). j-ranges are assigned in
# listed order. Rings: "hw" = SP HWDGE ring, "act" = ACT HWDGE ring,
# "sw" = SWDGE ring issued by the (otherwise idle-at-issue-time) Pool
# sequencer. The choreography keeps every DMA at <=1 explicit wait:
#  - each slice's FIRST dma needs its engine-sem wait, so it must sit on a
#    fresh DMA lane (8 HW lanes shared by hw+act rings, 8 SW lanes);
#  - later DMAs of a slice only ever carry one lane-capacity wait;
#  - per ring, wait ticks are monotone so the FIFO never blocks early data.
_cached = {}

SLICES = [
    # name, engine, ring, jn   (5 slices: 2 wait-levels per DMA ring max)
    ("d1", "vector", "hw", 20),
    ("d2a", "vector", "hw", 36),
    ("d2b", "vector", "sw", 64),
    ("a1", "scalar", "act", 56),
    ("p1", "gpsimd", "sw", 16),
]


def _slice_layout(slices):
    j0 = 0
    out = {}
    for name, eng, ring, jn in slices:
        out[name] = (eng, ring, j0, jn)
        j0 += jn
    assert j0 == W, j0
    return out


def _build_nc(slices=None, skip_dmas=False, skip_copies=False):
    import concourse.bass as bass
    import concourse.mybir as mybir
    import concourse.tile as tile
    from concourse.tile_rust import add_dep_helper
    from contextlib import ExitStack

    if slices is None:
        slices = SLICES
    layout = _slice_layout(slices)
    nc = bass.Bass("TRN2", target_bir_lowering=False)
    qel = nc.declare_dram_parameter("qel", [NR, QELFREE], mybir.dt.uint8, isOutput=False)
    out = nc.declare_dram_parameter("out", [NI * W * ROWBLK], mybir.dt.uint8, isOutput=True)

    with ExitStack() as ctx:
        tc = ctx.enter_context(tile.TileContext(nc))
        pool = ctx.enter_context(tc.tile_pool(name="p", bufs=1))
        qel_t = pool.tile([NR, QELFREE], mybir.dt.uint8)
        nc.sync.dma_start(out=qel_t[:], in_=qel.ap())

        l_tiles = {}
        # copies in slice order per engine (order within an engine stream
        # follows emission, which matches the DMA readiness order)
        for name, (ename, ring, j0, jn) in layout.items():
            e = getattr(nc, ename)
            lfree = jn * PBLK
            l_t = pool.tile([NR, lfree], mybir.dt.uint8, tag=f"l_{name}")
            l_tiles[name] = l_t
            if skip_copies:
                e.tensor_copy(
                    bass.AP(l_t.tensor, l_t.offset, [[lfree, NR], [1, 64]]),
                    bass.AP(qel_t.tensor, qel_t.offset, [[QELFREE, NR], [1, 64]]),
                ) if e is not nc.scalar else e.copy(
                    bass.AP(l_t.tensor, l_t.offset, [[lfree, NR], [1, 64]]),
                    bass.AP(qel_t.tensor, qel_t.offset, [[QELFREE, NR], [1, 64]]),
                )
                continue
            for u in range(2):
                prev_c = None
                for ty in range(TEMPLATE):
                    # DVE 2x_2P needs an even innermost dim: write 8-wide
                    # tx runs for ty<6; the spill byte lands on the next
                    # ty sub-block's tx=0 slot and is overwritten by the
                    # (order-enforced) ty+1 copy. ty=6 stays 7-wide (its
                    # spill would cross a dx block written earlier).
                    width = 8 if (ename == "vector" and ty < TEMPLATE - 1) else TEMPLATE
                    src = bass.AP(
                        qel_t.tensor,
                        qel_t.offset + (u + ty) * PADW + j0,
                        [[QELFREE, NR], [1, jn], [1, CV], [1, width]],
                    )
                    dst = bass.AP(
                        l_t.tensor,
                        l_t.offset + u * BLK + ty * TEMPLATE,
                        [[lfree, NR], [PBLK, jn], [TEMPLATE * TEMPLATE, CV], [1, width]],
                    )
                    if e is nc.scalar:
                        c = e.copy(dst, src)
                    else:
                        c = e.tensor_copy(dst, src)
                    if ename == "vector" and prev_c is not None:
                        add_dep_helper(c.ins, prev_c.ins, False, "spill-fix-order")
                    prev_c = c

        def dma_a(issuer, name, g):
            ename, ring, j0, jn = layout[name]
            lfree = jn * PBLK
            l_t = l_tiles[name]
            src = bass.AP(
                l_t.tensor,
                l_t.offset + 2 * g * lfree,
                [[lfree, NI], [PBLK, jn], [1, PBLK]],
            )
            dst = bass.AP(
                out,
                j0 * ROWBLK + g * PBLK,
                [[W * ROWBLK, NI], [ROWBLK, jn], [1, PBLK]],
            )
            return issuer.dma_start(out=dst, in_=src)

        def dma_b(issuer, name):
            ename, ring, j0, jn = layout[name]
            lfree = jn * PBLK
            l_t = l_tiles[name]
            src = bass.AP(
                l_t.tensor,
                l_t.offset + 6 * lfree,
                [[lfree, NI], [PBLK, jn], [1, BLK]],
            )
            dst = bass.AP(
                out,
                j0 * ROWBLK + 6 * BLK,
                [[W * ROWBLK, NI], [ROWBLK, jn], [1, BLK]],
            )
            return issuer.dma_start(out=dst, in_=src)

        def group(issuer, name):
            return [dma_a(issuer, name, 0), dma_a(issuer, name, 1),
                    dma_a(issuer, name, 2), dma_b(issuer, name)]

        dmas = []
        if not skip_dmas:
            hw_names = [n for n, (e, r, _, _) in layout.items() if r == "hw"]
            act_names = [n for n, (e, r, _, _) in layout.items() if r == "act"]
            sw_names = [n for n, (e, r, _, _) in layout.items() if r == "sw"]
            assert len(hw_names) == 2 and len(act_names) == 1, (hw_names, act_names)
            h1, h2 = hw_names
            (an,) = act_names
            swp = [n for n in sw_names if layout[n][0] == "gpsimd"]
            swd = [n for n in sw_names if layout[n][0] != "gpsimd"]
            assert len(swp) == 1 and len(swd) == 1, sw_names

            # HW-lane chain: load(0), h1 group(1-4), h2A0(5), a1A0(6),
            # h2A1(7) are the first 8 HW-counter DMAs (fresh lanes); the
            # rest carry single capacity waits. Ring orders stay monotone:
            # SP: load, h1*, h2*;  ACT: a1* (one wait level).
            h1g = group(nc.sync, h1)
            h2g = group(nc.sync, h2)
            a1g = group(nc.scalar, an)
            hw_chain = h1g + [h2g[0], a1g[0], h2g[1], h2g[2], h2g[3],
                              a1g[1], a1g[2], a1g[3]]
            for prev, d in zip(hw_chain, hw_chain[1:]):
                add_dep_helper(d.ins, prev.ins, False, "hw-lane-order")

            # SW-lane chain (all issued by the Pool sequencer): p1 group
            # (lanes 0-3) then the DVE-owned late slice (lanes 4-7).
            p1g = group(nc.gpsimd, swp[0])
            d2g = group(nc.gpsimd, swd[0])
            sw_chain = p1g + d2g
            for prev, d in zip(sw_chain, sw_chain[1:]):
                add_dep_helper(d.ins, prev.ins, False, "sw-lane-order")
            dmas = hw_chain + sw_chain

        # wait-carrier NoOps for the tail drain redistribution
        wait_nops = []
        for _ in range(22):
            nop = nc.sync.nop()
            if dmas:
                for d in (dmas[11], dmas[-1]):
                    add_dep_helper(nop.ins, d.ins, True, "tail-order")
            wait_nops.append(nop)

    _redistribute_tail_waits(nc, [n.ins for n in wait_nops])
    return nc


def _redistribute_tail_waits(nc, carrier_nops):
    """Walrus allows one explicit sync-wait per instruction; Tile's tail
    drain aggregates one wait per outstanding proc. Move the excess onto
    the dedicated NoOps that sit at the end of the SP stream."""
    import concourse.mybir as mybir

    carrier_names = {n.name for n in carrier_nops}
    multi = []
    for bb in nc.m.functions[0].blocks:
        for inst in bb.instructions:
            si = inst.sync_info
            if si is not None and si.on_wait and len(si.on_wait) > 1:
                if inst.name not in carrier_names:
                    multi.append(inst)
    if not multi:
        for nop in carrier_nops:
            if nop.sync_info is not None and nop.sync_info.on_wait:
                nop.sync_info.on_wait = nop.sync_info.on_wait[:1]
        return
    assert len(multi) == 1 and isinstance(multi[0], mybir.InstDrain), (
        "unexpected multi-wait instructions: "
        + ", ".join(f"{type(i).__name__}:{i.name}" for i in multi)
    )
    drain = multi[0]
    waits = list(drain.sync_info.on_wait)
    extra, keep = waits[:-1], waits[-1:]
    assert len(extra) <= len(carrier_nops), (len(extra), len(carrier_nops))
    for nop, w in zip(carrier_nops, extra):
        si = nop.sync_info
        if si is None:
            nop.sync_info = mybir.SyncInfo(on_wait=[w], on_update=[])
        else:
            si.on_wait = [w]
    for nop in carrier_nops[len(extra):]:
        if nop.sync_info is not None and nop.sync_info.on_wait:
            nop.sync_info.on_wait = nop.sync_info.on_wait[:1]
    drain.sync_info.on_wait = keep


def _host_prep(inputs):
    x = np.asarray(inputs)
    assert x.shape == (2, 1, H, W), x.shape
    q = np.floor(x[:, 0]).astype(np.uint8)
    qpad = np.zeros((2, H + 2 * PAD, PADW), np.uint8)
    qpad[:, PAD:PAD + H, PAD:PAD + W] = q
    in_maps = []
    for c in range(N_CORES):
        b = c // 4
        i0 = 48 * (c % 4)
        # rows i0 .. i0+NR-1+NWIN-1 = i0+60 may exceed the padded image for
        # the last i0; those rows are only read by the dead u=1 half of
        # L2[NR-1], so clamping is safe.
        idx = i0 + np.arange(NR)[:, None] + np.arange(NWIN)[None, :]
        idx = np.minimum(idx, H + 2 * PAD - 1)
        qel = qpad[b][idx].reshape(NR, QELFREE)
        in_maps.append({"qel": np.ascontiguousarray(qel)})
    return in_maps


def kernel(inputs, search_range):
    assert int(search_range) == MAX_SR, search_range
    from concourse.bass_utils import run_bass_kernel_spmd

    if "nc" not in _cached:
        _cached["nc"] = _build_nc()
    nc = _cached["nc"]
    in_maps = _host_prep(inputs)
    res = run_bass_kernel_spmd(nc, in_maps, list(range(N_CORES)))
    full = np.empty((2, H, W, CV * CV, TEMPLATE * TEMPLATE), np.uint8)
    for c in range(N_CORES):
        b = c // 4
        i0 = 48 * (c % 4)
        full[b, i0:i0 + NI] = res.results[c]["out"].reshape(NI, W, CV * CV, TEMPLATE * TEMPLATE)
    return full
